# revision 57
# baseline (speedup 1.0000x reference)
"""KNN-conv kernel for Trainium2, data-parallel over batch on 8 NeuronCores.

Problem: for x (32, 128, 32, 32) and conv weight W (128, 128, 9):
  per batch: cosine-sim (1024x1024) over channels, diag -> +INF, top-9
  neighbors per token, gather neighbor features, contract with W.

Per core (4 batches), software-pipelined ~2 batches deep. Design notes:
  - normalization: ||x||^2 per token via 8 tiny PE matmuls (transposed
    layout), sqrt/+eps/reciprocal on Act/DVE, DRAM bounce to token order,
    then a stride-0 partition-broadcast DMA fills the [128,1024] scale
    tile; xn = x * r on the Pool engine (keeps PE/DVE free)
  - similarity needs fp32-rank fidelity (top-k amplifies sim noise: f32r
    fails outright, 1-pass bf16/fp16 fail, bf16 hi+lo 3-pass doubles the
    error). KNN_SIM=tri (default) exploits symmetry: upper-triangle
    blocks via true-fp32 matmuls, lower blocks via exact PE transposes of
    the SBUF copies (the freshest block last so its copy has landed) --
    fp32 accuracy at ~25% less PE time. KNN_SIM=f32/hilo kept for tests.
  - diag forced to -1e10 with a tiny accumulating bf16 identity matmul;
    rank-0 neighbor is the token itself (self term taken from an on-chip
    fp16 cast of x -- no gather)
  - top-8 via DVE max8 + find-index8 per 128-token block: 32 pairs x
    1.13us is the hard bottleneck engine; everything else hides under it
  - the 16-wrap gather index layout is built ON-CHIP (the baseline's
    DRAM round-trip cost ~29us/batch of descriptor-limited DMA): u16->
    fp16 cast (Pool, off the bottleneck DVE stream), 8 selection matmuls
    (0/1 fp16, exact for idx<2048) replicate+permute partitions into
    PSUM, shuffled PSUM->SBUF copy emits uint16 indices (DVE)
  - neighbor gather via dma_gather(transpose=True) from token-major fp16
    rows (256B/token), one call per 2 c-blocks (256 tokens x 8 ranks;
    the last batch ends with two single-block calls to shorten the
    drain); rank-planes land contiguous, aligned with conv PSUM groups
  - conv = single-pass fp16 matmuls (vs the baseline's bf16-hi: same
    cost, 8x less quantization error) accumulated in fp32 PSUM; output
    stored per segment
  - fill tricks: batch 0's x load is issued before the const DMAs; its
    norm broadcast skips the DRAM bounce (PE transpose + matmul
    broadcast + DVE multiply, all idle engines at fill); activation
    tables preloaded while idle; X16 cast after SQ; prologues issued 2
    batches ahead so norm-bounce DMAs are not stuck behind gather DMAs

Cost-model time: 127054 ns (baseline 298847), HW rel err 0.00466.
"""

import os
from contextlib import ExitStack

import numpy as np

B, C, N, K = 32, 128, 1024, 9
O = 128  # out channels
NCORES = 8
BPC = B // NCORES  # batches per core
NEG = -1.0e10

_prog_cache = {}
last_results = None  # BassKernelResults of the most recent run (for test.py)
STAGE_MARKS = []  # (label, next-instruction-id) pairs when KNN_MARK=1


def _build_program():
    import concourse.bacc as bacc
    import concourse.mybir as mybir
    from concourse.tile import TileContext

    f32 = mybir.dt.float32
    bf16 = mybir.dt.bfloat16
    fp16 = mybir.dt.float16
    u16 = mybir.dt.uint16
    i16 = mybir.dt.int16
    AF = mybir.ActivationFunctionType
    Alu = mybir.AluOpType

    sim_mode = os.environ.get("KNN_SIM", "tri")

    nc = bacc.Bacc()

    x_h = nc.declare_dram_parameter("x", [BPC, C, N], f32, isOutput=False)
    xt_h = nc.declare_dram_parameter("xt", [BPC, N, C], fp16, isOutput=False)
    wt_h = nc.declare_dram_parameter("wt", [C, K * O], fp16, isOutput=False)
    ident_h = nc.declare_dram_parameter("ident", [128, 128], bf16, isOutput=False)
    identf_h = nc.declare_dram_parameter("identf", [128, 128], f32, isOutput=False)
    negi_h = nc.declare_dram_parameter("negi", [128, 128], bf16, isOutput=False)
    sel_h = nc.declare_dram_parameter("sel", [128, 8 * 128], fp16, isOutput=False)
    ones128_h = nc.declare_dram_parameter("ones128", [C, 1], f32, isOutput=False)
    e8_h = nc.declare_dram_parameter("e8", [8, 8 * 128], f32, isOutput=False)
    out_h = nc.declare_dram_parameter("out", [BPC, O, N], f32, isOutput=True)

    rd_h = nc.dram_tensor("rd", [BPC, N], f32)

    with TileContext(nc) as tc:
        with ExitStack() as es:
            pool = lambda name, bufs, **kw: es.enter_context(
                tc.tile_pool(name=name, bufs=bufs, **kw)
            )
            consts = pool("consts", 1)
            xp = pool("xp", 3)
            x16p = pool("x16p", 4)
            sqp = pool("sqp", 3)
            xnp = pool("xnp", 2)
            hp = pool("hp", 3)
            lp = pool("lp", 3)
            rsp = pool("rsp", 2)
            scp = pool("scp", 16)
            smallp = pool("smallp", 6)
            v8p = pool("v8p", 10)
            idxp = pool("idxp", 3)
            idxfp = pool("idxfp", 8)
            idxgp = pool("idxgp", 8)
            prp = pool("prp", 8)
            outp = pool("outp", 6)
            psA = pool("psA", 2, space="PSUM")
            psN = pool("psN", 1, space="PSUM")
            psT = pool("psT", 1, space="PSUM")
            psO = pool("psO", 2, space="PSUM")

            X0 = xp.tile([C, N], f32, tag="x")
            nc.sync.dma_start(out=X0[:], in_=x_h[0])

            wts = consts.tile([C, K * O], fp16, tag="wts")
            nc.sync.dma_start(out=wts[:], in_=wt_h[:])
            ident = consts.tile([128, 128], bf16, tag="ident")
            nc.sync.dma_start(out=ident[:], in_=ident_h[:])
            identf = consts.tile([128, 128], f32, tag="identf")
            nc.sync.dma_start(out=identf[:], in_=identf_h[:])
            negi = consts.tile([128, 128], bf16, tag="negi")
            nc.sync.dma_start(out=negi[:], in_=negi_h[:])
            sel = consts.tile([128, 8 * 128], fp16, tag="sel")
            nc.sync.dma_start(out=sel[:], in_=sel_h[:])
            ones128 = consts.tile([C, 1], f32, tag="ones128")
            nc.sync.dma_start(out=ones128[:], in_=ones128_h[:])
            e8 = consts.tile([8, 8 * 128], f32, tag="e8")
            nc.sync.dma_start(out=e8[:], in_=e8_h[:])
            # touch every activation function now so both ACT table sets
            # load while the engine is otherwise idle (saves ~1.3us on the
            # first batch's norm chain)
            warm = consts.tile([1, 8], f32, tag="warm")
            nc.vector.memset(warm[:], 1.0)
            warm2 = consts.tile([1, 8], f32, tag="warm2")
            nc.scalar.activation(warm2[:], warm[:], AF.Square)
            nc.scalar.activation(warm2[:], warm[:], AF.Sqrt)
            nc.scalar.copy(warm2[:], warm[:])

            st = [dict() for _ in range(BPC)]  # per-batch live tiles

            mark_on = bool(int(os.environ.get("KNN_MARK", "0")))

            def mark(label):
                if mark_on:
                    STAGE_MARKS.append((label, nc.next_id()))

            def prologue(b):
                s = st[b]
                if b == 0:
                    X = X0
                else:
                    X = xp.tile([C, N], f32, tag="x")
                    nc.sync.dma_start(out=X[:], in_=x_h[b])
                SQ = sqp.tile([C, N], f32, tag="sq")
                nc.scalar.activation(SQ[:], X[:], AF.Square)
                X16 = x16p.tile([C, N], fp16, tag="x16")
                nc.scalar.copy(X16[:], X[:])
                # norm^2 transposed: n2[p, blk] = sum_c SQ[c, blk*128+p]
                n2 = psN.tile([128, 8], f32, tag="n2")
                for blk in range(8):
                    nc.tensor.matmul(
                        n2[:, blk : blk + 1],
                        SQ[:, blk * 128 : (blk + 1) * 128],
                        ones128[:],
                        start=True,
                        stop=True,
                    )
                sq8 = smallp.tile([128, 8], f32, tag="sq8")
                nc.scalar.activation(sq8[:], n2[:], AF.Sqrt)
                rA = smallp.tile([128, 8], f32, tag="rA")
                nc.vector.tensor_scalar_add(rA[:], sq8[:], 1e-8)
                rT = smallp.tile([128, 8], f32, tag="rT")
                nc.vector.reciprocal(rT[:], rA[:])
                if b > 0:
                    # bounce (128, 8) -> token-ordered (1, 1024) via DRAM
                    nc.sync.dma_start(
                        out=rd_h[b].rearrange("(blk p) -> p blk", p=128),
                        in_=rT[:],
                    )
                s["X"], s["X16"], s["rT"] = X, X16, rT

            def normalize(b):
                s = st[b]
                if b == 0:
                    # fill path: no DRAM round-trip. PE-transpose the per-
                    # block reciprocal norms, matmul-broadcast them across
                    # partitions, multiply on the (idle) DVE. ~4us shorter
                    # chain to the first sim block.
                    rTT = psT.tile([8, 128], f32, tag="pt")
                    nc.tensor.transpose(rTT[:], s["rT"][:], identf[:])
                    rT8 = smallp.tile([8, 128], f32, tag="rt8")
                    nc.scalar.copy(rT8[:], rTT[:])
                    R = psA.tile([128, 1024], f32, tag="ps_big")
                    for blk in range(8):
                        nc.tensor.matmul(
                            R[:, blk * 128 : (blk + 1) * 128],
                            e8[:, blk * 128 : (blk + 1) * 128],
                            rT8[:],
                            start=True,
                            stop=True,
                        )
                    XN = xnp.tile([C, N], f32, tag="xn")
                    nc.vector.tensor_mul(XN[:], s["X"], R[:])
                    s["XN"] = XN
                    return
                # broadcast r over partitions with a stride-0 DMA re-read of
                # the bounced token-order reciprocal norms; keeps PE out of
                # the normalization chain entirely
                RS = rsp.tile([128, 1024], f32, tag="rs")
                nc.sync.dma_start(
                    out=RS[:],
                    in_=rd_h[b]
                    .rearrange("(one n) -> one n", one=1)
                    .partition_broadcast(128),
                )
                XN = xnp.tile([C, N], f32, tag="xn")
                nc.gpsimd.tensor_mul(XN[:], s["X"], RS[:])
                s["XN"] = XN

            def prep_hilo(b):
                s = st[b]
                XN = s["XN"]
                if sim_mode == "hilo":
                    H = hp.tile([C, N], bf16, tag="h")
                    nc.scalar.copy(H[:], XN[:])
                    L = lp.tile([C, N], bf16, tag="l")
                    nc.gpsimd.tensor_tensor(L[:], XN[:], H[:], Alu.subtract)
                    s["H"], s["L"] = H, L

            def sim_block(b, c):
                s = st[b]
                IDX = s.get("IDX")
                if IDX is None:
                    IDX = idxp.tile([128, 64], u16, tag="idx")
                    s["IDX"] = IDX
                ps = psA.tile([128, 1024], f32, tag="ps_big")
                if sim_mode == "tri":
                    XN = s["XN"]
                    lhs = XN[:, c * 128 : (c + 1) * 128]
                    # lower-triangle blocks: exact PE transposes of the
                    # symmetric upper blocks already sitting in SBUF SC
                    # tiles. The freshest one (d=c-1) is transposed LAST so
                    # its Act copy has certainly landed by then.
                    for d in range(c - 1):
                        nc.tensor.transpose(
                            ps[:, d * 128 : (d + 1) * 128],
                            s["SC%d" % d][:, c * 128 : (c + 1) * 128],
                            identf[:],
                        )
                    # upper triangle incl. the diagonal block via matmuls
                    if c < 4:
                        nc.tensor.matmul(
                            ps[:, c * 128 : 512],
                            lhs,
                            XN[:, c * 128 : 512],
                            start=True,
                            stop=False,
                        )
                        nc.tensor.matmul(
                            ps[:, 512:], lhs, XN[:, 512:], start=True, stop=True
                        )
                    else:
                        nc.tensor.matmul(
                            ps[:, c * 128 :],
                            lhs,
                            XN[:, c * 128 :],
                            start=True,
                            stop=False,
                        )
                    # diag block -> -1e10 (accumulate -1e10*I)
                    nc.tensor.matmul(
                        ps[:, c * 128 : c * 128 + 128],
                        ident[:],
                        negi[:],
                        start=False,
                        stop=True,
                    )
                    if c >= 1:
                        d = c - 1
                        nc.tensor.transpose(
                            ps[:, d * 128 : (d + 1) * 128],
                            s["SC%d" % d][:, c * 128 : (c + 1) * 128],
                            identf[:],
                        )
                    SC = scp.tile([128, N], f32, tag="sc")
                    nc.scalar.copy(SC[:], ps[:])
                    s["SC%d" % c] = SC
                    V8 = v8p.tile([128, 8], f32, tag="v8")
                    nc.vector.max(V8[:], SC[:])
                    nc.vector.max_index(IDX[:, c : 64 : 8], V8[:], SC[:])
                    return
                if sim_mode == "hilo":
                    H, L = s["H"], s["L"]
                    Hc = H[:, c * 128 : (c + 1) * 128]
                    Lc = L[:, c * 128 : (c + 1) * 128]
                    for half in range(2):
                        cols = slice(512 * half, 512 * (half + 1))
                        last = (c < 4) if half else (c >= 4)
                        nc.tensor.matmul(
                            ps[:, cols], Hc, H[:, cols], start=True, stop=False
                        )
                        nc.tensor.matmul(
                            ps[:, cols], Hc, L[:, cols], start=False, stop=False
                        )
                        nc.tensor.matmul(
                            ps[:, cols], Lc, H[:, cols], start=False, stop=last
                        )
                else:
                    XN = s["XN"]
                    lhs = XN[:, c * 128 : (c + 1) * 128]
                    nc.tensor.matmul(
                        ps[:, :512], lhs, XN[:, :512], start=True, stop=(c >= 4)
                    )
                    nc.tensor.matmul(
                        ps[:, 512:], lhs, XN[:, 512:], start=True, stop=(c < 4)
                    )
                # diag block -> -1e10 (accumulate -1e10*I)
                nc.tensor.matmul(
                    ps[:, c * 128 : c * 128 + 128],
                    ident[:],
                    negi[:],
                    start=False,
                    stop=True,
                )
                SC = scp.tile([128, N], f32, tag="sc")
                nc.scalar.copy(SC[:], ps[:])
                V8 = v8p.tile([128, 8], f32, tag="v8")
                nc.vector.max(V8[:], SC[:])
                # rank-major layout IDX[p, 8j+c]
                nc.vector.max_index(IDX[:, c : 64 : 8], V8[:], SC[:])

            def tail_idx_gather(b, c_lo, c_cnt=2):
                """IDX -> 16-wrap gather layout on-chip, launch the gather
                for c_cnt c-blocks starting at c_lo (c_cnt*128 tokens x 8
                ranks). IDX[16sl+q', 8j+(c_lo+cc)] ->
                IDXG[16g+q', (j*c_cnt+cc)*8+sl]."""
                s = st[b]
                IDX = s["IDX"]
                nidx = c_cnt * 128 * 8
                IDXF = idxfp.tile([128, 8 * c_cnt], fp16, tag="idxf%d" % c_cnt)
                nc.gpsimd.tensor_copy(
                    IDXF[:].rearrange("p (j cc) -> p j cc", j=8),
                    IDX[:].rearrange("p (j c) -> p j c", j=8)[
                        :, :, c_lo : c_lo + c_cnt
                    ],
                )
                PT = psT.tile([128, 8, 16], f32, tag="pt")
                for sl in range(8):
                    nc.tensor.matmul(
                        PT[:, sl, 0 : 8 * c_cnt],
                        sel[:, sl * 128 : (sl + 1) * 128],
                        IDXF[:],
                        start=True,
                        stop=True,
                    )
                IDXG = idxgp.tile([128, nidx // 16], u16, tag="idxg%d" % c_cnt)
                nc.vector.tensor_copy(
                    IDXG[:].rearrange(
                        "p (j cc sl) -> p j cc sl", j=8, cc=c_cnt
                    ),
                    PT[:, :, 0 : 8 * c_cnt].rearrange(
                        "p sl (j cc) -> p j cc sl", j=8
                    ),
                )
                PR = prp.tile([128, nidx], fp16, tag="pr%d" % c_cnt)
                nc.gpsimd.dma_gather(
                    out_ap=PR[:].rearrange("p (t n) -> p t n", t=1),
                    in_ap=xt_h[b],
                    idxs_ap=IDXG[:].bitcast(i16),
                    num_idxs=nidx,
                    num_idxs_reg=nidx,
                    elem_size=C,
                    transpose=True,
                    single_packet=False,
                )
                s["PR%d" % c_lo] = (PR, c_cnt)

            def tail_conv(b, c_lo):
                s = st[b]
                PR, c_cnt = s.pop("PR%d" % c_lo)
                w = 128 * c_cnt
                h = c_lo // 4
                sub = slice(128 * c_lo - 512 * h, 128 * c_lo - 512 * h + w)
                PO = s.get("PO%d" % h)
                if PO is None:
                    PO = psO.tile([O, 512], f32, tag="ps_out")
                    s["PO%d" % h] = PO
                for k in range(9):
                    if k == 0:
                        src = s["X16"][:, 128 * c_lo : 128 * c_lo + w]
                    else:
                        src = PR[:, (k - 1) * w : k * w]
                    nc.tensor.matmul(
                        PO[:, sub],
                        wts[:, k * O : (k + 1) * O],
                        src,
                        start=(k == 0),
                        stop=(k == 8),
                    )
                OUT = outp.tile([O, w], f32, tag="out%d" % c_cnt)
                nc.scalar.copy(OUT[:], PO[:, sub])
                nc.sync.dma_start(
                    out=out_h[b][:, 128 * c_lo : 128 * c_lo + w], in_=OUT[:]
                )

            # ---- software-pipelined issue order -------------------------
            # PE/Act/DVE in-order queues: every op with a cross-engine
            # dependency (sel matmuls after DVE max_index; conv after the
            # gather DMA) is issued late enough that its deps are already
            # resolved when the queue reaches it, keeping the tensor engine
            # continuously busy (its p-state ramp resets on any idle gap).
            # Convs lag their gathers by most of an iteration.
            ORD = os.environ.get("KNN_ORD", "A")
            mark("prologue0"); prologue(0)
            mark("normalize0"); normalize(0)
            mark("hilo0"); prep_hilo(0)
            mark("sim0c0"); sim_block(0, 0)
            mark("sim0c1"); sim_block(0, 1)
            mark("prologue1"); prologue(1)
            mark("sim0c2"); sim_block(0, 2)
            mark("sim0c3"); sim_block(0, 3)
            for b in range(BPC):
                nb, nnb = b + 1, b + 2
                if nnb < BPC:
                    mark("prologue%d" % nnb); prologue(nnb)
                if nb < BPC:
                    mark("normalize%d" % nb); normalize(nb)
                if ORD == "A":
                    last = b == BPC - 1
                    mark("sim%dc4" % b); sim_block(b, 4)
                    mark("gather%dc0" % b); tail_idx_gather(b, 0, 2)
                    mark("sim%dc5" % b); sim_block(b, 5)
                    mark("gather%dc2" % b); tail_idx_gather(b, 2, 2)
                    if nb < BPC:
                        mark("hilo%d" % nb); prep_hilo(nb)
                    mark("sim%dc6" % b); sim_block(b, 6)
                    mark("gather%dc4" % b); tail_idx_gather(b, 4, 2)
                    mark("sim%dc7" % b); sim_block(b, 7)
                    if last:
                        mark("gather%dc6" % b); tail_idx_gather(b, 6, 1)
                        mark("gather%dc7" % b); tail_idx_gather(b, 7, 1)
                    if nb < BPC:
                        mark("sim%dc0" % nb); sim_block(nb, 0)
                    if not last:
                        mark("gather%dc6" % b); tail_idx_gather(b, 6, 2)
                    if nb < BPC:
                        for c in range(1, 4):
                            mark("sim%dc%d" % (nb, c)); sim_block(nb, c)
                    if b > 0:
                        for c_lo in (0, 2, 4, 6):
                            mark("conv%dc%d" % (b - 1, c_lo))
                            tail_conv(b - 1, c_lo)
                elif ORD == "B":
                    mark("sim%dc4" % b); sim_block(b, 4)
                    if b > 0:
                        mark("conv%dq0" % (b - 1)); tail_conv(b - 1, 0)
                        mark("conv%dq1" % (b - 1)); tail_conv(b - 1, 2)
                    mark("sim%dc5" % b); sim_block(b, 5)
                    mark("gather%dq0" % b); tail_idx_gather(b, 0)
                    if nb < BPC:
                        mark("hilo%d" % nb); prep_hilo(nb)
                    mark("sim%dc6" % b); sim_block(b, 6)
                    if b > 0:
                        mark("conv%dq2" % (b - 1)); tail_conv(b - 1, 4)
                        mark("conv%dq3" % (b - 1)); tail_conv(b - 1, 6)
                    mark("gather%dq1" % b); tail_idx_gather(b, 2)
                    mark("sim%dc7" % b); sim_block(b, 7)
                    mark("gather%dq2" % b); tail_idx_gather(b, 4)
                    if nb < BPC:
                        mark("sim%dc0" % nb); sim_block(nb, 0)
                    mark("gather%dq3" % b); tail_idx_gather(b, 6)
                    if nb < BPC:
                        for c in range(1, 4):
                            mark("sim%dc%d" % (nb, c)); sim_block(nb, c)
                else:  # "C": convs first, gathers interleaved
                    if b > 0:
                        for q in range(4):
                            mark("conv%dq%d" % (b - 1, q)); tail_conv(b - 1, q)
                    mark("sim%dc4" % b); sim_block(b, 4)
                    mark("gather%dq0" % b); tail_idx_gather(b, 0)
                    mark("sim%dc5" % b); sim_block(b, 5)
                    mark("gather%dq1" % b); tail_idx_gather(b, 2)
                    if nb < BPC:
                        mark("hilo%d" % nb); prep_hilo(nb)
                    mark("sim%dc6" % b); sim_block(b, 6)
                    mark("sim%dc7" % b); sim_block(b, 7)
                    mark("gather%dq2" % b); tail_idx_gather(b, 4)
                    if nb < BPC:
                        mark("sim%dc0" % nb); sim_block(nb, 0)
                    mark("gather%dq3" % b); tail_idx_gather(b, 6)
                    if nb < BPC:
                        for c in range(1, 4):
                            mark("sim%dc%d" % (nb, c)); sim_block(nb, c)
            b = BPC - 1
            for c_lo in (0, 2, 4, 6, 7):
                mark("conv%dc%d" % (b, c_lo)); tail_conv(b, c_lo)
            mark("end")

    nc.compile()
    return nc


def _get_program():
    if "nc" not in _prog_cache:
        _prog_cache["nc"] = _build_program()
    return _prog_cache["nc"]


def _host_prep(x, W):
    """Build per-core input maps from full inputs."""
    import ml_dtypes

    fp16 = np.float16
    xf = np.ascontiguousarray(x.reshape(B, C, N).astype(np.float32, copy=False))
    # token-major fp16 rows, 256B per token
    xt = np.ascontiguousarray(xf.transpose(0, 2, 1).astype(fp16))

    wt = np.ascontiguousarray(
        np.transpose(W.astype(np.float32, copy=False), (1, 2, 0))
    ).reshape(C, K * O).astype(fp16)

    ident = np.eye(128, dtype=ml_dtypes.bfloat16)
    identf = np.eye(128, dtype=np.float32)
    negi = (NEG * np.eye(128, dtype=np.float32)).astype(ml_dtypes.bfloat16)
    ones128 = np.ones((C, 1), dtype=np.float32)
    e8 = np.zeros((8, 8 * 128), dtype=np.float32)
    for blk in range(8):
        e8[blk, blk * 128 : (blk + 1) * 128] = 1.0

    # selection matrices: sel[:, sl*128:(sl+1)*128][16sl+q, 16g+q] = 1
    sel = np.zeros((128, 8, 128), dtype=fp16)
    for sl in range(8):
        for g in range(8):
            for q in range(16):
                sel[16 * sl + q, sl, 16 * g + q] = 1.0
    sel = sel.reshape(128, 8 * 128)

    in_maps = []
    for i in range(NCORES):
        sl_ = slice(i * BPC, (i + 1) * BPC)
        in_maps.append(
            {
                "x": np.ascontiguousarray(xf[sl_]),
                "xt": np.ascontiguousarray(xt[sl_]),
                "wt": wt,
                "ident": ident,
                "identf": identf,
                "negi": negi,
                "sel": sel,
                "ones128": ones128,
                "e8": e8,
            }
        )
    return in_maps


def kernel(x, W):
    global last_results
    from concourse.bass_utils import run_bass_kernel_spmd

    x = np.asarray(x)
    W = np.asarray(W)
    in_maps = _host_prep(x, W)
    nc = _get_program()
    trace = bool(int(os.environ.get("KNN_TRACE", "0")))
    res = run_bass_kernel_spmd(nc, in_maps, list(range(NCORES)), trace=trace)
    last_results = res
    out = np.concatenate([res.results[i]["out"] for i in range(NCORES)], axis=0)
    return out.reshape(B, O, 32, 32).astype(np.float32, copy=False)
